# revision 7
# baseline (speedup 1.0000x reference)
"""Trainium2 Bass kernel for NetTGCN (gnn_message_passing) — v2.

The Chebyshev SpMMs are dense matmuls against a densified normalized
adjacency LhatT (fp16, built on host from edge_index).  LhatT is
column-sharded across the 8 cores: each core holds lhsT [N, N/8] = 16 MB
SBUF-resident and computes its 1024 output rows per application.

v2 restructure vs v1:
 * Column-major SpMM: the z state tile [128 src-nodes, F<=128] is the
   STATIONARY matmul operand (one weight load per contraction tile) and
   the resident LhatT is the MOVING operand (2x512-wide).  The SpMM
   output is zT [F, 1024] which feeds the per-tap weight GEMMs directly
   (features already on partitions) — no per-tap transposes on the tap
   GEMM path and 8x fewer LDWEIGHTS on the SpMM path.
 * Two independent batch-split chains per conv (8 batches each).  Each
   chain's AllGather flies while the other chain's SpMM runs on the PE.
 * Gathered z is streamed from the DRAM gather buffer in [128, 8, F]
   super-tiles (double buffered) instead of a full SBUF copy.
 * The node-major copy of T_k needed for the gather is rebuilt with 8 PE
   transposes per chain/chunk/tap into one PSUM bank.

The FFT (real part of a length-15 DFT) is a constant cosine matrix folded
into W1 on the host.  fc1 is contraction-sharded (each core streams 1/8
of the weight from HBM) with a final AllReduce; fc2 + log_softmax run
replicated.  All matmul operands fp16 with fp32 PSUM accumulation.
"""

import numpy as np

# ---------------------------------------------------------------- config

class CFG:
    N = 8192          # nodes
    B = 16            # batch
    T = 15            # time taps
    KCH = 25          # chebyshev order
    G1 = 32
    G2 = 64
    C = 512           # fc1 out
    D = 6             # classes
    NCORES = 8
    NCH = 2           # batch chains
    PHASES = 3        # 1=conv1, 2=+conv2, 3=+fc (debug bisect)
    DEBUG = False

    @property
    def NLOC(self):
        return self.N // self.NCORES

    @property
    def MT(self):
        return self.NLOC // 128

    @property
    def KT(self):
        return self.N // 128

    @property
    def BCH(self):
        return self.B // self.NCH        # batches per chain (8)

    @property
    def F1(self):
        return self.BCH * self.T         # conv1 chain width (120)

    @property
    def F2(self):
        return self.BCH * self.G1        # conv2 chain width (256)


def _host_prep(cfg, x, edge_index, W1, b1, W2, b2, fc1_w, fc1_b, fc2_w, fc2_b):
    """Pure layout / format preprocessing -> per-core input maps."""
    f16 = np.float16
    N, B, T, K = cfg.N, cfg.B, cfg.T, cfg.KCH
    NC, NLOC, MT, KT = cfg.NCORES, cfg.NLOC, cfg.MT, cfg.KT
    G1, G2 = cfg.G1, cfg.G2

    row = np.asarray(edge_index[0], dtype=np.int64)
    col = np.asarray(edge_index[1], dtype=np.int64)
    deg = np.bincount(row, minlength=N).astype(np.float32)
    dinv = np.where(deg > 0, 1.0 / np.sqrt(np.maximum(deg, 1.0)), 0.0).astype(np.float32)
    vals = -(dinv[row] * dinv[col])
    # convention: LhatT[r, c] such that out[c] += LhatT[r, c] * z[r]
    LhatT = np.zeros((N, N), np.float32)
    np.add.at(LhatT, (row, col), vals)
    LhatT = LhatT.astype(f16)

    # x -> [p, kt, (b,t)] fp16, node n = kt*128 + p
    x_n = np.ascontiguousarray(
        np.asarray(x, np.float32).transpose(1, 0, 2).reshape(KT, 128, B * T)
        .transpose(1, 0, 2)).astype(f16)

    # fold DFT-real (cosine) matrix into W1:  xf = x @ Cf ; W1f[k] = Cf @ W1[k]
    tt = np.arange(T)
    Cf = np.cos(2 * np.pi * np.outer(tt, tt) / T).astype(np.float32)
    W1f = np.einsum('ts,ksg->ktg', Cf, np.asarray(W1, np.float32))  # [K, T, G1]

    # block-diag over the 8 batches of one chain -> sbuf [F1, K, 256]
    W1blk = np.zeros((K, cfg.F1, cfg.BCH * G1), np.float32)
    for b8 in range(cfg.BCH):
        W1blk[:, b8 * T:(b8 + 1) * T, b8 * G1:(b8 + 1) * G1] = W1f
    W1blk = np.ascontiguousarray(W1blk.transpose(1, 0, 2)).astype(f16)

    # block-diag over 4 batches of one chunk -> sbuf [128, K, 256]
    W2blk = np.zeros((K, 4 * G1, 4 * G2), np.float32)
    for b4 in range(4):
        W2blk[:, b4 * G1:(b4 + 1) * G1, b4 * G2:(b4 + 1) * G2] = \
            np.asarray(W2, np.float32)
    W2blk = np.ascontiguousarray(W2blk.transpose(1, 0, 2)).astype(f16)

    b1row = np.tile(np.asarray(b1, np.float32), B)[None, :].astype(f16)   # [1, 512]
    b2row = np.tile(np.asarray(b2, np.float32), B)[None, :].astype(f16)   # [1, 1024]
    ones_col = np.ones((1, 128), f16)
    fc1b_row = np.asarray(fc1_b, np.float32)[None, :].astype(f16)         # [1, C]
    fc2_wT = np.ascontiguousarray(
        np.asarray(fc2_w, np.float32).T.reshape(cfg.C // 128, 128, cfg.D)
        .transpose(1, 0, 2))                                              # [128, C/128, D] f32
    fc2b_col = np.asarray(fc2_b, np.float32)[None, :]                     # [1, D]
    ones_f32 = np.ones((1, cfg.B), np.float32)

    wv = np.asarray(fc1_w, np.float32).reshape(cfg.C, N, G2)
    xt = np.asarray(x, np.float32).transpose(1, 0, 2)                     # [N, B, T]

    in_maps = []
    for c in range(NC):
        # LhatT column slice -> [p, kt, mt, m] fp16  (r = kt*128+p)
        lt = LhatT[:, c * NLOC:(c + 1) * NLOC]
        lt = np.ascontiguousarray(
            lt.reshape(KT, 128, MT, 128).transpose(1, 0, 2, 3))
        # local x transposed per chain: xT[ch][(b,t), n_loc] fp16
        xl = xt[c * NLOC:(c + 1) * NLOC]                                  # [NLOC, B, T]
        xT = np.ascontiguousarray(
            xl.reshape(NLOC, cfg.NCH, cfg.BCH * T).transpose(1, 2, 0)).astype(f16)
        # fc1 weight slice -> [p, jt, cc] with jt = g*MT + mt, j = jt*128 + p
        ws = wv[:, c * NLOC:(c + 1) * NLOC, :]                            # [C, NLOC, G2]
        ws = ws.reshape(cfg.C, MT, 128, G2).transpose(2, 3, 1, 0)         # [p, g, mt, C]
        ws = np.ascontiguousarray(ws.reshape(128, G2 * MT, cfg.C)).astype(f16)
        in_maps.append(dict(
            lt=lt, x_n=x_n, x_t=xT,
            w1blk=W1blk, w2blk=W2blk, b1row=b1row, b2row=b2row,
            ones16=ones_col, fc1b=fc1b_row, fc2wt=fc2_wT, fc2b=fc2b_col,
            onesf32=ones_f32, wfc=ws,
        ))
    return in_maps


def _build(cfg):
    import concourse.bass as bass
    import concourse.mybir as mybir
    import concourse.tile as tile
    from concourse import bacc
    from concourse.masks import make_identity

    f16 = mybir.dt.float16
    f32 = mybir.dt.float32
    AT = mybir.ActivationFunctionType
    OP = mybir.AluOpType
    AX = mybir.AxisListType

    N, B, T, K = cfg.N, cfg.B, cfg.T, cfg.KCH
    NC, NLOC, MT, KT = cfg.NCORES, cfg.NLOC, cfg.MT, cfg.KT
    NCH, BCH, F1, F2 = cfg.NCH, cfg.BCH, cfg.F1, cfg.F2
    G1, G2, C, D = cfg.G1, cfg.G2, cfg.C, cfg.D
    RG = [list(range(NC))]
    KTG = KT // 8                       # kt super-tile groups (8)

    nc = bacc.Bacc("TRN2", target_bir_lowering=False, debug=False,
                   num_devices=NC)

    dt_in = {
        'lt': ([128, KT, MT, 128], f16),
        'x_n': ([128, KT, B * T], f16),
        'x_t': ([NCH, F1, NLOC], f16),
        'w1blk': ([F1, K, BCH * G1], f16),
        'w2blk': ([4 * G1, K, 4 * G2], f16),
        'b1row': ([1, B * G1], f16),
        'b2row': ([1, B * G2], f16),
        'ones16': ([1, 128], f16),
        'fc1b': ([1, C], f16),
        'fc2wt': ([128, C // 128, D], f32),
        'fc2b': ([1, D], f32),
        'onesf32': ([1, B], f32),
        'wfc': ([128, G2 * MT, C], f16),
    }
    din = {k: nc.dram_tensor(k, shp, dt, kind="ExternalInput").ap()
           for k, (shp, dt) in dt_in.items()}
    dout = nc.dram_tensor("out", [B, D], f32, kind="ExternalOutput").ap()
    if cfg.DEBUG:
        dbg_h1 = nc.dram_tensor("dbg_h1", [128, MT, B * G1], f16,
                                kind="ExternalOutput").ap()
        dbg_h2 = nc.dram_tensor("dbg_h2", [128, MT, B * G2], f16,
                                kind="ExternalOutput").ap()

    with tile.TileContext(nc) as tc:
        with (
            tc.tile_pool(name="const", bufs=1) as constp,
            tc.tile_pool(name="dram", bufs=1, space="DRAM") as dramp,
        ):
            # ---------------- constants / persistent state
            LT = constp.tile([128, KT, MT, 128], f16)
            for g in range(8):
                nc.sync.dma_start(LT[:, g * 8:(g + 1) * 8],
                                  din['lt'][:, g * 8:(g + 1) * 8])
            ident16 = constp.tile([128, 128], f16)
            make_identity(nc, ident16[:])
            identf32 = constp.tile([32, 32], f32)
            make_identity(nc, identf32[:])
            ones16 = constp.tile([1, 128], f16)
            nc.sync.dma_start(ones16[:], din['ones16'])

            # DRAM gather buffers (2 tap-parity bufs per chain)
            def gbufs(name, fdim):
                gis = [dramp.tile([MT, 128, fdim], f16, name=f"{name}i{i}")
                       for i in range(2)]
                gos = [dramp.tile([KT, 128, fdim], f16, name=f"{name}o{i}")
                       for i in range(2)]
                return gis, gos

            g1 = [gbufs(f"g1c{ch}", F1) for ch in range(NCH)]
            g2 = [gbufs(f"g2c{ch}", F2) for ch in range(NCH)]
            gh1_i = [dramp.tile([MT, 128, BCH * G1], f16, name=f"gh1i{ch}")
                     for ch in range(NCH)]
            gh1_o = [dramp.tile([KT, 128, BCH * G1], f16, name=f"gh1o{ch}")
                     for ch in range(NCH)]

            # persistent relu'd conv outputs
            accp = tc.tile_pool(name="accp", bufs=1)
            accpp = accp.__enter__()
            h1loc = accpp.tile([128, MT, B * G1], f16)
            acc2 = accpp.tile([128, MT, B * G2], f16)

            # =========================================================
            # generic chebyshev conv driver (column-major SpMM)
            # =========================================================
            def conv(tag, nq, chF, wblk, brow, zsrc_fn, ztinit_fn,
                     acc, acc_col0_fn, gbuf, pools):
                """One Chebyshev conv: NCH chains x nq chunks of width chF.

                wblk: SBUF AP [chF(+), K, 256]; tap k chunk rhs = wblk[:chF, k, :]
                zsrc_fn(kk, ch, g): DRAM AP [128, 8, chF*nq] = SpMM input
                    super-tile (T_{kk-1} node-major, contraction group g).
                ztinit_fn(ch, q, zt): init zt [chF, NLOC] = chunk of T_0^T.
                acc_col0_fn(ch, q): starting acc column of chunk (ch, q).
                """
                zsp, pszp, pstp, psgp, ztp, curp = pools
                OW = 256
                FW = chF * nq                   # chain width
                zt = [[[ztp.tile([chF, NLOC], f16,
                                 name=f"zt{tag}_{ch}_{q}_{par}")
                        for par in range(2)] for q in range(nq)]
                      for ch in range(NCH)]
                for ch in range(NCH):
                    for q in range(nq):
                        ztinit_fn(ch, q, zt[ch][q][0])

                # k = 0 tap GEMM from the initial state
                for ch in range(NCH):
                    for q in range(nq):
                        c0 = acc_col0_fn(ch, q)
                        for mt in range(MT):
                            pg = psgp.tile([128, OW], f32, tag="pg")
                            nc.tensor.matmul(
                                pg[:], zt[ch][q][0][:, mt * 128:(mt + 1) * 128],
                                wblk[:chF, 0, :], start=True, stop=True)
                            nc.vector.tensor_tensor(
                                acc[:, mt, c0:c0 + OW], acc[:, mt, c0:c0 + OW],
                                pg[:], OP.add)

                for kk in range(1, K):
                    par, prev = kk % 2, (kk - 2) % 2
                    for ch in range(NCH):
                        # ---- SpMM: psT[f, c] = sum_r z[r, f] * LhatT[r, c]
                        psts = [pszp.tile([chF, MT * 128], f32, tag="psz",
                                           name=f"psz{tag}_{kk}_{ch}_{q}")
                                for q in range(nq)]
                        for g in range(KTG):
                            zs = zsp.tile([128, 8, FW], f16, tag="zs")
                            nc.sync.dma_start(zs[:], zsrc_fn(kk, ch, g))
                            for k8 in range(8):
                                kt = g * 8 + k8
                                for q in range(nq):
                                    lhs = zs[:, k8, q * chF:(q + 1) * chF]
                                    nc.tensor.matmul(
                                        psts[q][:, 0:512], lhs,
                                        LT[:, kt, 0:4, :],
                                        start=(kt == 0), stop=(kt == KT - 1))
                                    nc.tensor.matmul(
                                        psts[q][:, 512:1024], lhs,
                                        LT[:, kt, 4:8, :],
                                        start=(kt == 0), stop=(kt == KT - 1))
                        # ---- evict: zt_k = 2*psT - zt_{k-2}   (T_1 = psT)
                        for q in range(nq):
                            dst = zt[ch][q][par]
                            if kk == 1:
                                nc.vector.tensor_copy(dst[:], psts[q][:])
                            else:
                                nc.vector.scalar_tensor_tensor(
                                    dst[:], psts[q][:], 2.0,
                                    zt[ch][q][prev][:], OP.mult, OP.subtract)
                        # ---- rebuild node-major + gather (skip last tap)
                        if kk < K - 1:
                            cur = curp.tile([128, MT, FW], f16, tag="cur")
                            for q in range(nq):
                                pt = pstp.tile([128, MT, chF], f16, tag="pst")
                                for mt in range(MT):
                                    nc.tensor.transpose(
                                        pt[:, mt, :],
                                        zt[ch][q][par][:, mt * 128:(mt + 1) * 128],
                                        ident16[:chF, :chF])
                                nc.vector.tensor_copy(
                                    cur[:, :, q * chF:(q + 1) * chF], pt[:])
                            gi, go = gbuf[ch][0][kk % 2], gbuf[ch][1][kk % 2]
                            nc.sync.dma_start(
                                gi[:].rearrange("m p f -> p m f"), cur[:])
                            nc.gpsimd.collective_compute(
                                "AllGather", OP.bypass, replica_groups=RG,
                                ins=[gi[:]], outs=[go[:]])
                        # ---- tap GEMMs
                        last = (kk == K - 1)
                        for q in range(nq):
                            c0 = acc_col0_fn(ch, q)
                            for mt in range(MT):
                                pg = psgp.tile([128, OW], f32, tag="pg")
                                nc.tensor.matmul(
                                    pg[:],
                                    zt[ch][q][par][:, mt * 128:(mt + 1) * 128],
                                    wblk[:chF, kk, :], start=True,
                                    stop=not last)
                                if last:
                                    nc.tensor.matmul(
                                        pg[:], ones16[:1, :128],
                                        brow[:1, c0:c0 + OW],
                                        start=False, stop=True)
                                nc.vector.tensor_tensor(
                                    acc[:, mt, c0:c0 + OW],
                                    acc[:, mt, c0:c0 + OW], pg[:], OP.add)

            # =========================================================
            # conv1: 2 chains x 1 chunk of F1=120
            # =========================================================
            with (
                tc.tile_pool(name="c1sb", bufs=1) as c1sbp,
                tc.tile_pool(name="zs1", bufs=2) as zs1p,
                tc.tile_pool(name="zt1", bufs=1) as zt1p,
                tc.tile_pool(name="cur1", bufs=2) as cur1p,
                tc.tile_pool(name="psz1", bufs=2, space="PSUM") as psz1p,
                tc.tile_pool(name="pst1", bufs=2, space="PSUM") as pst1p,
                tc.tile_pool(name="psg1", bufs=2, space="PSUM") as psg1p,
            ):
                w1 = c1sbp.tile([F1, K, BCH * G1], f16)
                nc.sync.dma_start(w1[:], din['w1blk'])
                b1row = c1sbp.tile([1, B * G1], f16)
                nc.sync.dma_start(b1row[:], din['b1row'])
                acc1 = c1sbp.tile([128, MT, B * G1], f32)
                nc.vector.memset(acc1[:], 0.0)

                def zsrc1(kk, ch, g):
                    if kk == 1:   # T_0 = x, node-major from x_n input
                        return din['x_n'][:, g * 8:(g + 1) * 8,
                                          ch * F1:(ch + 1) * F1]
                    go = g1[ch][1][(kk - 1) % 2]
                    return go[g * 8:(g + 1) * 8].rearrange("k p f -> p k f")

                def ztinit1(ch, q, ztile):
                    nc.sync.dma_start(ztile[:], din['x_t'][ch])

                with nc.named_scope("conv1"):
                    conv("c1", 1, F1, w1, b1row, zsrc1, ztinit1,
                         acc1, lambda ch, q: ch * (BCH * G1), g1,
                         (zs1p, psz1p, pst1p, psg1p, zt1p, cur1p))

                    # h1 = relu(acc1), per-chain gather to all cores
                    HW1 = BCH * G1
                    for ch in range(NCH):
                        cs = slice(ch * HW1, (ch + 1) * HW1)
                        nc.vector.tensor_scalar_max(
                            h1loc[:, :, cs], acc1[:, :, cs], 0.0)
                        nc.sync.dma_start(
                            gh1_i[ch][:].rearrange("m p f -> p m f"),
                            h1loc[:, :, cs])
                        nc.gpsimd.collective_compute(
                            "AllGather", OP.bypass, replica_groups=RG,
                            ins=[gh1_i[ch][:]], outs=[gh1_o[ch][:]])
            if cfg.DEBUG:
                nc.sync.dma_start(dbg_h1, h1loc[:])

            if cfg.PHASES < 2:
                zz = constp.tile([B, D], f32)
                nc.vector.memset(zz[:], 0.0)
                nc.sync.dma_start(dout, zz[:])
                accp.__exit__(None, None, None)
                return nc

            # =========================================================
            # conv2: 2 chains x 2 chunks of 128
            # =========================================================
            with (
                tc.tile_pool(name="c2sb", bufs=1) as c2sbp,
                tc.tile_pool(name="zs2", bufs=2) as zs2p,
                tc.tile_pool(name="zt2", bufs=1) as zt2p,
                tc.tile_pool(name="cur2", bufs=2) as cur2p,
                tc.tile_pool(name="psz2", bufs=2, space="PSUM") as psz2p,
                tc.tile_pool(name="pst2", bufs=2, space="PSUM") as pst2p,
                tc.tile_pool(name="psg2", bufs=2, space="PSUM") as psg2p,
            ):
                w2 = c2sbp.tile([4 * G1, K, 4 * G2], f16)
                nc.sync.dma_start(w2[:], din['w2blk'])
                b2row = c2sbp.tile([1, B * G2], f16)
                nc.sync.dma_start(b2row[:], din['b2row'])
                nc.vector.memset(acc2[:], 0.0)

                def zsrc2(kk, ch, g):
                    if kk == 1:   # T_0 = h1 full, from the per-chain gather
                        return gh1_o[ch][g * 8:(g + 1) * 8] \
                            .rearrange("k p f -> p k f")
                    go = g2[ch][1][(kk - 1) % 2]
                    return go[g * 8:(g + 1) * 8].rearrange("k p f -> p k f")

                def ztinit2(ch, q, ztile):
                    # zt = (h1 chunk)^T via PE transposes of h1loc columns
                    f0 = ch * F2 + q * 128
                    pt = pst2p.tile([128, MT, 128], f16, tag="pst")
                    for mt in range(MT):
                        nc.tensor.transpose(
                            pt[:, mt, :], h1loc[:, mt, f0:f0 + 128],
                            ident16[:])
                    nc.vector.tensor_copy(
                        ztile[:].rearrange("p (m f) -> p m f", m=MT), pt[:])

                with nc.named_scope("conv2"):
                    conv("c2", 2, 128, w2, b2row, zsrc2, ztinit2,
                         acc2, lambda ch, q: ch * (BCH * G2) + q * 256, g2,
                         (zs2p, psz2p, pst2p, psg2p, zt2p, cur2p))
                    nc.vector.tensor_scalar_max(acc2[:], acc2[:], 0.0)
            if cfg.DEBUG:
                nc.sync.dma_start(dbg_h2, acc2[:])

            if cfg.PHASES < 3:
                zz = constp.tile([B, D], f32)
                nc.vector.memset(zz[:], 0.0)
                nc.sync.dma_start(dout, zz[:])
                accp.__exit__(None, None, None)
                return nc

            # =========================================================
            # fc1 (streamed weights, contraction-sharded) + fc2 + lsm
            # =========================================================
            h2v = acc2[:].rearrange("p m (b g) -> p m b g", b=B)
            with (
                nc.named_scope("fc"),
                tc.tile_pool(name="fcw", bufs=3) as fcwp,
                tc.tile_pool(name="fcps", bufs=1, space="PSUM") as fcpsp,
                tc.tile_pool(name="fcsb", bufs=1) as fcsbp,
                tc.tile_pool(name="fcps2", bufs=2, space="PSUM") as fcps2p,
            ):
                JT = G2 * MT            # 512 j-tiles
                JBLK = 8
                psfc = fcpsp.tile([B, C], f32)
                fc1b_sb = fcsbp.tile([1, C], f16)
                nc.sync.dma_start(fc1b_sb[:], din['fc1b'])
                for jb in range(JT // JBLK):
                    wbuf = fcwp.tile([128, JBLK, C], f16, tag="wbuf")
                    nc.sync.dma_start(
                        wbuf[:], din['wfc'][:, jb * JBLK:(jb + 1) * JBLK, :])
                    for ji in range(JBLK):
                        jt = jb * JBLK + ji
                        g, mt = jt // MT, jt % MT
                        nc.tensor.matmul(psfc[:], h2v[:, mt, :, g],
                                         wbuf[:, ji, :],
                                         start=(jt == 0), stop=False)
                nc.tensor.matmul(psfc[:], ones16[:1, :B], fc1b_sb[:1, :],
                                 start=False, stop=True)

                # transpose [B, C] -> [128, C/128, B]
                hsb = fcsbp.tile([B, C], f32)
                nc.vector.tensor_copy(hsb[:], psfc[:])
                hT = fcsbp.tile([128, C // 128, B], f32)
                for t4 in range(C // 128):
                    tp = fcps2p.tile([128, B], f32, tag="fct")
                    nc.tensor.transpose(tp[:], hsb[:, t4 * 128:(t4 + 1) * 128],
                                        identf32[:B, :B])
                    nc.vector.tensor_copy(hT[:, t4, :], tp[:])

                arin = dramp.tile([128, C // 128, B], f32)
                arout = dramp.tile([128, C // 128, B], f32)
                nc.sync.dma_start(arin[:], hT[:])
                nc.gpsimd.collective_compute(
                    "AllReduce", OP.add, replica_groups=RG,
                    ins=[arin[:]], outs=[arout[:]])
                hTr = fcsbp.tile([128, C // 128, B], f32)
                nc.sync.dma_start(hTr[:], arout[:])

                # fc2: out[d, b] = fc2_w[d, :] @ h[:, b]
                fc2wt = fcsbp.tile([128, C // 128, D], f32)
                nc.sync.dma_start(fc2wt[:], din['fc2wt'])
                fc2b = fcsbp.tile([1, D], f32)
                nc.sync.dma_start(fc2b[:], din['fc2b'])
                onesf32 = fcsbp.tile([1, B], f32)
                nc.sync.dma_start(onesf32[:], din['onesf32'])
                ps2 = fcps2p.tile([D, B], f32, tag="ps2")
                for kt in range(C // 128):
                    nc.tensor.matmul(ps2[:], fc2wt[:, kt, :], hTr[:, kt, :],
                                     start=(kt == 0), stop=False)
                nc.tensor.matmul(ps2[:], fc2b[:1, :], onesf32[:1, :],
                                 start=False, stop=True)

                s2 = fcsbp.tile([D, B], f32)
                nc.vector.tensor_copy(s2[:], ps2[:])
                ps3 = fcps2p.tile([B, D], f32, tag="ps3")
                nc.tensor.transpose(ps3[:], s2[:], identf32[:D, :D])
                sm = fcsbp.tile([B, D], f32)
                nc.vector.tensor_copy(sm[:], ps3[:])

                # log_softmax over D (free axis)
                mx = fcsbp.tile([B, 1], f32)
                nc.vector.tensor_reduce(mx[:], sm[:], AX.X, OP.max)
                xm = fcsbp.tile([B, D], f32)
                nc.vector.tensor_single_scalar(xm[:], sm[:], mx[:], OP.subtract)
                ex = fcsbp.tile([B, D], f32)
                nc.scalar.activation(ex[:], xm[:], AT.Exp)
                sume = fcsbp.tile([B, 1], f32)
                nc.vector.tensor_reduce(sume[:], ex[:], AX.X, OP.add)
                lse = fcsbp.tile([B, 1], f32)
                nc.scalar.activation(lse[:], sume[:], AT.Ln)
                res = fcsbp.tile([B, D], f32)
                nc.vector.tensor_single_scalar(res[:], xm[:], lse[:],
                                               OP.subtract)
                nc.sync.dma_start(dout, res[:])
            accp.__exit__(None, None, None)

    return nc


def _run(cfg, inputs, trace=False):
    in_maps = _host_prep(cfg, **inputs)
    nc = _build(cfg)
    nc.compile()
    from concourse import bass_utils
    res = bass_utils.run_bass_kernel_spmd(
        nc, in_maps, core_ids=list(range(cfg.NCORES)), trace=trace)
    return np.asarray(res.results[0]['out'], np.float32).copy(), res


def kernel(**inputs):
    out, _ = _run(CFG(), inputs)
    return out


# revision 8
# speedup vs baseline: 1.0173x; 1.0173x over previous
"""Trainium2 Bass kernel for NetTGCN (gnn_message_passing) — v2.

The Chebyshev SpMMs are dense matmuls against a densified normalized
adjacency LhatT (fp16, built on host from edge_index).  LhatT is
column-sharded across the 8 cores: each core holds lhsT [N, N/8] = 16 MB
SBUF-resident and computes its 1024 output rows per application.

v2 restructure vs v1:
 * Column-major SpMM: the z state tile [128 src-nodes, F<=128] is the
   STATIONARY matmul operand (one weight load per contraction tile) and
   the resident LhatT is the MOVING operand (2x512-wide).  The SpMM
   output is zT [F, 1024] which feeds the per-tap weight GEMMs directly
   (features already on partitions) — no per-tap transposes on the tap
   GEMM path and 8x fewer LDWEIGHTS on the SpMM path.
 * Two independent batch-split chains per conv (8 batches each).  Each
   chain's AllGather flies while the other chain's SpMM runs on the PE.
 * Gathered z is streamed from the DRAM gather buffer in [128, 8, F]
   super-tiles (double buffered) instead of a full SBUF copy.
 * The node-major copy of T_k needed for the gather is rebuilt with 8 PE
   transposes per chain/chunk/tap into one PSUM bank.

The FFT (real part of a length-15 DFT) is a constant cosine matrix folded
into W1 on the host.  fc1 is contraction-sharded (each core streams 1/8
of the weight from HBM) with a final AllReduce; fc2 + log_softmax run
replicated.  All matmul operands fp16 with fp32 PSUM accumulation.
"""

import numpy as np

# ---------------------------------------------------------------- config

class CFG:
    N = 8192          # nodes
    B = 16            # batch
    T = 15            # time taps
    KCH = 25          # chebyshev order
    G1 = 32
    G2 = 64
    C = 512           # fc1 out
    D = 6             # classes
    NCORES = 8
    NCH = 2           # batch chains
    PHASES = 3        # 1=conv1, 2=+conv2, 3=+fc (debug bisect)
    DEBUG = False

    @property
    def NLOC(self):
        return self.N // self.NCORES

    @property
    def MT(self):
        return self.NLOC // 128

    @property
    def KT(self):
        return self.N // 128

    @property
    def BCH(self):
        return self.B // self.NCH        # batches per chain (8)

    @property
    def F1(self):
        return self.BCH * self.T         # conv1 chain width (120)

    @property
    def F2(self):
        return self.BCH * self.G1        # conv2 chain width (256)


def _host_prep(cfg, x, edge_index, W1, b1, W2, b2, fc1_w, fc1_b, fc2_w, fc2_b):
    """Pure layout / format preprocessing -> per-core input maps."""
    f16 = np.float16
    N, B, T, K = cfg.N, cfg.B, cfg.T, cfg.KCH
    NC, NLOC, MT, KT = cfg.NCORES, cfg.NLOC, cfg.MT, cfg.KT
    G1, G2 = cfg.G1, cfg.G2

    row = np.asarray(edge_index[0], dtype=np.int64)
    col = np.asarray(edge_index[1], dtype=np.int64)
    deg = np.bincount(row, minlength=N).astype(np.float32)
    dinv = np.where(deg > 0, 1.0 / np.sqrt(np.maximum(deg, 1.0)), 0.0).astype(np.float32)
    vals = -(dinv[row] * dinv[col])
    # convention: LhatT[r, c] such that out[c] += LhatT[r, c] * z[r]
    LhatT = np.zeros((N, N), np.float32)
    np.add.at(LhatT, (row, col), vals)
    LhatT = LhatT.astype(f16)

    # x -> [p, kt, (b,t)] fp16, node n = kt*128 + p
    x_n = np.ascontiguousarray(
        np.asarray(x, np.float32).transpose(1, 0, 2).reshape(KT, 128, B * T)
        .transpose(1, 0, 2)).astype(f16)

    # fold DFT-real (cosine) matrix into W1:  xf = x @ Cf ; W1f[k] = Cf @ W1[k]
    tt = np.arange(T)
    Cf = np.cos(2 * np.pi * np.outer(tt, tt) / T).astype(np.float32)
    W1f = np.einsum('ts,ksg->ktg', Cf, np.asarray(W1, np.float32))  # [K, T, G1]

    # block-diag over the 8 batches of one chain -> sbuf [F1, K, 256]
    W1blk = np.zeros((K, cfg.F1, cfg.BCH * G1), np.float32)
    for b8 in range(cfg.BCH):
        W1blk[:, b8 * T:(b8 + 1) * T, b8 * G1:(b8 + 1) * G1] = W1f
    W1blk = np.ascontiguousarray(W1blk.transpose(1, 0, 2)).astype(f16)

    # block-diag over 4 batches of one chunk -> sbuf [128, K, 256]
    W2blk = np.zeros((K, 4 * G1, 4 * G2), np.float32)
    for b4 in range(4):
        W2blk[:, b4 * G1:(b4 + 1) * G1, b4 * G2:(b4 + 1) * G2] = \
            np.asarray(W2, np.float32)
    W2blk = np.ascontiguousarray(W2blk.transpose(1, 0, 2)).astype(f16)

    b1row = np.tile(np.asarray(b1, np.float32), B)[None, :].astype(f16)   # [1, 512]
    b2row = np.tile(np.asarray(b2, np.float32), B)[None, :].astype(f16)   # [1, 1024]
    ones_col = np.ones((1, 128), f16)
    fc1b_row = np.asarray(fc1_b, np.float32)[None, :].astype(f16)         # [1, C]
    fc2_wT = np.ascontiguousarray(
        np.asarray(fc2_w, np.float32).T.reshape(cfg.C // 128, 128, cfg.D)
        .transpose(1, 0, 2))                                              # [128, C/128, D] f32
    fc2b_col = np.asarray(fc2_b, np.float32)[None, :]                     # [1, D]
    ones_f32 = np.ones((1, cfg.B), np.float32)

    wv = np.asarray(fc1_w, np.float32).reshape(cfg.C, N, G2)
    xt = np.asarray(x, np.float32).transpose(1, 0, 2)                     # [N, B, T]

    in_maps = []
    for c in range(NC):
        # LhatT column slice -> [p, kt, mt, m] fp16  (r = kt*128+p)
        lt = LhatT[:, c * NLOC:(c + 1) * NLOC]
        lt = np.ascontiguousarray(
            lt.reshape(KT, 128, MT, 128).transpose(1, 0, 2, 3))
        # local x transposed per chain: xT[ch][(b,t), n_loc] fp16
        xl = xt[c * NLOC:(c + 1) * NLOC]                                  # [NLOC, B, T]
        xT = np.ascontiguousarray(
            xl.reshape(NLOC, cfg.NCH, cfg.BCH * T).transpose(1, 2, 0)).astype(f16)
        # fc1 weight slice -> [p, jt, cc] with jt = g*MT + mt, j = jt*128 + p
        ws = wv[:, c * NLOC:(c + 1) * NLOC, :]                            # [C, NLOC, G2]
        ws = ws.reshape(cfg.C, MT, 128, G2).transpose(2, 3, 1, 0)         # [p, g, mt, C]
        ws = np.ascontiguousarray(ws.reshape(128, G2 * MT, cfg.C)).astype(f16)
        in_maps.append(dict(
            lt=lt, x_n=x_n, x_t=xT,
            w1blk=W1blk, w2blk=W2blk, b1row=b1row, b2row=b2row,
            ones16=ones_col, fc1b=fc1b_row, fc2wt=fc2_wT, fc2b=fc2b_col,
            onesf32=ones_f32, wfc=ws,
        ))
    return in_maps


def _build(cfg):
    import concourse.bass as bass
    import concourse.mybir as mybir
    import concourse.tile as tile
    from concourse import bacc
    from concourse.masks import make_identity

    f16 = mybir.dt.float16
    f32 = mybir.dt.float32
    AT = mybir.ActivationFunctionType
    OP = mybir.AluOpType
    AX = mybir.AxisListType

    N, B, T, K = cfg.N, cfg.B, cfg.T, cfg.KCH
    NC, NLOC, MT, KT = cfg.NCORES, cfg.NLOC, cfg.MT, cfg.KT
    NCH, BCH, F1, F2 = cfg.NCH, cfg.BCH, cfg.F1, cfg.F2
    G1, G2, C, D = cfg.G1, cfg.G2, cfg.C, cfg.D
    RG = [list(range(NC))]
    KTG = KT // 8                       # kt super-tile groups (8)

    nc = bacc.Bacc("TRN2", target_bir_lowering=False, debug=False,
                   num_devices=NC)

    dt_in = {
        'lt': ([128, KT, MT, 128], f16),
        'x_n': ([128, KT, B * T], f16),
        'x_t': ([NCH, F1, NLOC], f16),
        'w1blk': ([F1, K, BCH * G1], f16),
        'w2blk': ([4 * G1, K, 4 * G2], f16),
        'b1row': ([1, B * G1], f16),
        'b2row': ([1, B * G2], f16),
        'ones16': ([1, 128], f16),
        'fc1b': ([1, C], f16),
        'fc2wt': ([128, C // 128, D], f32),
        'fc2b': ([1, D], f32),
        'onesf32': ([1, B], f32),
        'wfc': ([128, G2 * MT, C], f16),
    }
    din = {k: nc.dram_tensor(k, shp, dt, kind="ExternalInput").ap()
           for k, (shp, dt) in dt_in.items()}
    dout = nc.dram_tensor("out", [B, D], f32, kind="ExternalOutput").ap()
    if cfg.DEBUG:
        dbg_h1 = nc.dram_tensor("dbg_h1", [128, MT, B * G1], f16,
                                kind="ExternalOutput").ap()
        dbg_h2 = nc.dram_tensor("dbg_h2", [128, MT, B * G2], f16,
                                kind="ExternalOutput").ap()

    with tile.TileContext(nc) as tc:
        with (
            tc.tile_pool(name="const", bufs=1) as constp,
            tc.tile_pool(name="dram", bufs=1, space="DRAM") as dramp,
        ):
            # ---------------- constants / persistent state
            LT = constp.tile([128, KT, MT, 128], f16)
            for g in range(8):
                nc.sync.dma_start(LT[:, g * 8:(g + 1) * 8],
                                  din['lt'][:, g * 8:(g + 1) * 8])
            ident16 = constp.tile([128, 128], f16)
            make_identity(nc, ident16[:])
            identf32 = constp.tile([32, 32], f32)
            make_identity(nc, identf32[:])
            ones16 = constp.tile([1, 128], f16)
            nc.sync.dma_start(ones16[:], din['ones16'])

            # DRAM gather buffers (2 tap-parity bufs per chain)
            def gbufs(name, fdim):
                gis = [dramp.tile([MT, 128, fdim], f16, name=f"{name}i{i}")
                       for i in range(2)]
                gos = [dramp.tile([KT, 128, fdim], f16, name=f"{name}o{i}")
                       for i in range(2)]
                return gis, gos

            g1 = [gbufs(f"g1c{ch}", F1) for ch in range(NCH)]
            g2 = [gbufs(f"g2c{ch}", F2) for ch in range(NCH)]
            gh1_i = [dramp.tile([MT, 128, BCH * G1], f16, name=f"gh1i{ch}")
                     for ch in range(NCH)]
            gh1_o = [dramp.tile([KT, 128, BCH * G1], f16, name=f"gh1o{ch}")
                     for ch in range(NCH)]

            # persistent relu'd conv outputs
            accp = tc.tile_pool(name="accp", bufs=1)
            accpp = accp.__enter__()
            h1loc = accpp.tile([128, MT, B * G1], f16)
            acc2 = accpp.tile([128, MT, B * G2], f16)

            # =========================================================
            # generic chebyshev conv driver (column-major SpMM)
            # =========================================================
            def conv(tag, nq, chF, wblk, brow, zsrc_fn, ztinit_fn,
                     acc, acc_col0_fn, gbuf, pools):
                """One Chebyshev conv: NCH chains x nq chunks of width chF.

                wblk: SBUF AP [chF(+), K, 256]; tap k chunk rhs = wblk[:chF, k, :]
                zsrc_fn(kk, ch, g): DRAM AP [128, 8, chF*nq] = SpMM input
                    super-tile (T_{kk-1} node-major, contraction group g).
                ztinit_fn(ch, q, zt): init zt [chF, NLOC] = chunk of T_0^T.
                acc_col0_fn(ch, q): starting acc column of chunk (ch, q).
                """
                zsp, pszp, pstp, psgp, ztp, curp = pools
                OW = 256
                FW = chF * nq                   # chain width
                zt = [[[ztp.tile([chF, NLOC], f16,
                                 name=f"zt{tag}_{ch}_{q}_{par}")
                        for par in range(2)] for q in range(nq)]
                      for ch in range(NCH)]
                for ch in range(NCH):
                    for q in range(nq):
                        ztinit_fn(ch, q, zt[ch][q][0])

                # k = 0 tap GEMM from the initial state
                for ch in range(NCH):
                    for q in range(nq):
                        c0 = acc_col0_fn(ch, q)
                        for m2 in range(MT // 2):
                            pg = psgp.tile([128, 2, OW], f32, tag="pg",
                                           name=f"pg{tag}0_{ch}_{q}_{m2}")
                            for i in range(2):
                                mt = 2 * m2 + i
                                nc.tensor.matmul(
                                    pg[:, i, :],
                                    zt[ch][q][0][:, mt * 128:(mt + 1) * 128],
                                    wblk[:chF, 0, :], start=True, stop=True)
                            nc.vector.tensor_tensor(
                                acc[:, 2 * m2:2 * m2 + 2, c0:c0 + OW],
                                acc[:, 2 * m2:2 * m2 + 2, c0:c0 + OW],
                                pg[:], OP.add)

                for kk in range(1, K):
                    par, prev = kk % 2, (kk - 2) % 2
                    for ch in range(NCH):
                        # ---- SpMM: psT[f, c] = sum_r z[r, f] * LhatT[r, c]
                        psts = [pszp.tile([chF, MT * 128], f32, tag="psz",
                                           name=f"psz{tag}_{kk}_{ch}_{q}")
                                for q in range(nq)]
                        for g in range(KTG):
                            zs = zsp.tile([128, 8, FW], f16, tag="zs")
                            nc.scalar.dma_start(zs[:], zsrc_fn(kk, ch, g))
                            for k8 in range(8):
                                kt = g * 8 + k8
                                for q in range(nq):
                                    lhs = zs[:, k8, q * chF:(q + 1) * chF]
                                    nc.tensor.matmul(
                                        psts[q][:, 0:512], lhs,
                                        LT[:, kt, 0:4, :],
                                        start=(kt == 0), stop=(kt == KT - 1))
                                    nc.tensor.matmul(
                                        psts[q][:, 512:1024], lhs,
                                        LT[:, kt, 4:8, :],
                                        start=(kt == 0), stop=(kt == KT - 1))
                        # ---- evict: zt_k = 2*psT - zt_{k-2}   (T_1 = psT)
                        for q in range(nq):
                            dst = zt[ch][q][par]
                            if kk == 1:
                                nc.vector.tensor_copy(dst[:], psts[q][:])
                            else:
                                nc.vector.scalar_tensor_tensor(
                                    dst[:], psts[q][:], 2.0,
                                    zt[ch][q][prev][:], OP.mult, OP.subtract)
                        # ---- rebuild node-major + gather (skip last tap)
                        if kk < K - 1:
                            cur = curp.tile([128, MT, FW], f16, tag="cur")
                            for q in range(nq):
                                pt = pstp.tile([128, MT, chF], f16, tag="pst")
                                for mt in range(MT):
                                    nc.tensor.transpose(
                                        pt[:, mt, :],
                                        zt[ch][q][par][:, mt * 128:(mt + 1) * 128],
                                        ident16[:chF, :chF])
                                nc.vector.tensor_copy(
                                    cur[:, :, q * chF:(q + 1) * chF], pt[:])
                            gi, go = gbuf[ch][0][kk % 2], gbuf[ch][1][kk % 2]
                            nc.sync.dma_start(
                                gi[:].rearrange("m p f -> p m f"), cur[:])
                            nc.gpsimd.collective_compute(
                                "AllGather", OP.bypass, replica_groups=RG,
                                ins=[gi[:]], outs=[go[:]])
                        # ---- tap GEMMs
                        last = (kk == K - 1)
                        for q in range(nq):
                            c0 = acc_col0_fn(ch, q)
                            for m2 in range(MT // 2):
                                pg = psgp.tile([128, 2, OW], f32, tag="pg",
                                               name=f"pg{tag}_{kk}_{ch}_{q}_{m2}")
                                for i in range(2):
                                    mt = 2 * m2 + i
                                    nc.tensor.matmul(
                                        pg[:, i, :],
                                        zt[ch][q][par][:, mt * 128:(mt + 1) * 128],
                                        wblk[:chF, kk, :], start=True,
                                        stop=not last)
                                    if last:
                                        nc.tensor.matmul(
                                            pg[:, i, :], ones16[:1, :128],
                                            brow[:1, c0:c0 + OW],
                                            start=False, stop=True)
                                nc.vector.tensor_tensor(
                                    acc[:, 2 * m2:2 * m2 + 2, c0:c0 + OW],
                                    acc[:, 2 * m2:2 * m2 + 2, c0:c0 + OW],
                                    pg[:], OP.add)

            # =========================================================
            # conv1: 2 chains x 1 chunk of F1=120
            # =========================================================
            with (
                tc.tile_pool(name="c1sb", bufs=1) as c1sbp,
                tc.tile_pool(name="zs1", bufs=2) as zs1p,
                tc.tile_pool(name="zt1", bufs=1) as zt1p,
                tc.tile_pool(name="cur1", bufs=2) as cur1p,
                tc.tile_pool(name="psz1", bufs=2, space="PSUM") as psz1p,
                tc.tile_pool(name="pst1", bufs=2, space="PSUM") as pst1p,
                tc.tile_pool(name="psg1", bufs=2, space="PSUM") as psg1p,
            ):
                w1 = c1sbp.tile([F1, K, BCH * G1], f16)
                nc.sync.dma_start(w1[:], din['w1blk'])
                b1row = c1sbp.tile([1, B * G1], f16)
                nc.sync.dma_start(b1row[:], din['b1row'])
                acc1 = c1sbp.tile([128, MT, B * G1], f32)
                nc.vector.memset(acc1[:], 0.0)

                def zsrc1(kk, ch, g):
                    if kk == 1:   # T_0 = x, node-major from x_n input
                        return din['x_n'][:, g * 8:(g + 1) * 8,
                                          ch * F1:(ch + 1) * F1]
                    go = g1[ch][1][(kk - 1) % 2]
                    return go[g * 8:(g + 1) * 8].rearrange("k p f -> p k f")

                def ztinit1(ch, q, ztile):
                    nc.sync.dma_start(ztile[:], din['x_t'][ch])

                with nc.named_scope("conv1"):
                    conv("c1", 1, F1, w1, b1row, zsrc1, ztinit1,
                         acc1, lambda ch, q: ch * (BCH * G1), g1,
                         (zs1p, psz1p, pst1p, psg1p, zt1p, cur1p))

                    # h1 = relu(acc1), per-chain gather to all cores
                    HW1 = BCH * G1
                    for ch in range(NCH):
                        cs = slice(ch * HW1, (ch + 1) * HW1)
                        nc.vector.tensor_scalar_max(
                            h1loc[:, :, cs], acc1[:, :, cs], 0.0)
                        nc.sync.dma_start(
                            gh1_i[ch][:].rearrange("m p f -> p m f"),
                            h1loc[:, :, cs])
                        nc.gpsimd.collective_compute(
                            "AllGather", OP.bypass, replica_groups=RG,
                            ins=[gh1_i[ch][:]], outs=[gh1_o[ch][:]])
            if cfg.DEBUG:
                nc.sync.dma_start(dbg_h1, h1loc[:])

            if cfg.PHASES < 2:
                zz = constp.tile([B, D], f32)
                nc.vector.memset(zz[:], 0.0)
                nc.sync.dma_start(dout, zz[:])
                accp.__exit__(None, None, None)
                return nc

            # =========================================================
            # conv2: 2 chains x 2 chunks of 128
            # =========================================================
            with (
                tc.tile_pool(name="c2sb", bufs=1) as c2sbp,
                tc.tile_pool(name="zs2", bufs=2) as zs2p,
                tc.tile_pool(name="zt2", bufs=1) as zt2p,
                tc.tile_pool(name="cur2", bufs=2) as cur2p,
                tc.tile_pool(name="psz2", bufs=2, space="PSUM") as psz2p,
                tc.tile_pool(name="pst2", bufs=2, space="PSUM") as pst2p,
                tc.tile_pool(name="psg2", bufs=2, space="PSUM") as psg2p,
            ):
                w2 = c2sbp.tile([4 * G1, K, 4 * G2], f16)
                nc.sync.dma_start(w2[:], din['w2blk'])
                b2row = c2sbp.tile([1, B * G2], f16)
                nc.sync.dma_start(b2row[:], din['b2row'])
                nc.vector.memset(acc2[:], 0.0)

                def zsrc2(kk, ch, g):
                    if kk == 1:   # T_0 = h1 full, from the per-chain gather
                        return gh1_o[ch][g * 8:(g + 1) * 8] \
                            .rearrange("k p f -> p k f")
                    go = g2[ch][1][(kk - 1) % 2]
                    return go[g * 8:(g + 1) * 8].rearrange("k p f -> p k f")

                def ztinit2(ch, q, ztile):
                    # zt = (h1 chunk)^T via PE transposes of h1loc columns
                    f0 = ch * F2 + q * 128
                    pt = pst2p.tile([128, MT, 128], f16, tag="pst")
                    for mt in range(MT):
                        nc.tensor.transpose(
                            pt[:, mt, :], h1loc[:, mt, f0:f0 + 128],
                            ident16[:])
                    nc.vector.tensor_copy(
                        ztile[:].rearrange("p (m f) -> p m f", m=MT), pt[:])

                with nc.named_scope("conv2"):
                    conv("c2", 2, 128, w2, b2row, zsrc2, ztinit2,
                         acc2, lambda ch, q: ch * (BCH * G2) + q * 256, g2,
                         (zs2p, psz2p, pst2p, psg2p, zt2p, cur2p))
                    nc.vector.tensor_scalar_max(acc2[:], acc2[:], 0.0)
            if cfg.DEBUG:
                nc.sync.dma_start(dbg_h2, acc2[:])

            if cfg.PHASES < 3:
                zz = constp.tile([B, D], f32)
                nc.vector.memset(zz[:], 0.0)
                nc.sync.dma_start(dout, zz[:])
                accp.__exit__(None, None, None)
                return nc

            # =========================================================
            # fc1 (streamed weights, contraction-sharded) + fc2 + lsm
            # =========================================================
            h2v = acc2[:].rearrange("p m (b g) -> p m b g", b=B)
            with (
                nc.named_scope("fc"),
                tc.tile_pool(name="fcw", bufs=3) as fcwp,
                tc.tile_pool(name="fcps", bufs=1, space="PSUM") as fcpsp,
                tc.tile_pool(name="fcsb", bufs=1) as fcsbp,
                tc.tile_pool(name="fcps2", bufs=2, space="PSUM") as fcps2p,
            ):
                JT = G2 * MT            # 512 j-tiles
                JBLK = 8
                psfc = fcpsp.tile([B, C], f32)
                fc1b_sb = fcsbp.tile([1, C], f16)
                nc.sync.dma_start(fc1b_sb[:], din['fc1b'])
                for jb in range(JT // JBLK):
                    wbuf = fcwp.tile([128, JBLK, C], f16, tag="wbuf")
                    nc.sync.dma_start(
                        wbuf[:], din['wfc'][:, jb * JBLK:(jb + 1) * JBLK, :])
                    for ji in range(JBLK):
                        jt = jb * JBLK + ji
                        g, mt = jt // MT, jt % MT
                        nc.tensor.matmul(psfc[:], h2v[:, mt, :, g],
                                         wbuf[:, ji, :],
                                         start=(jt == 0), stop=False)
                nc.tensor.matmul(psfc[:], ones16[:1, :B], fc1b_sb[:1, :],
                                 start=False, stop=True)

                # transpose [B, C] -> [128, C/128, B]
                hsb = fcsbp.tile([B, C], f32)
                nc.vector.tensor_copy(hsb[:], psfc[:])
                hT = fcsbp.tile([128, C // 128, B], f32)
                for t4 in range(C // 128):
                    tp = fcps2p.tile([128, B], f32, tag="fct")
                    nc.tensor.transpose(tp[:], hsb[:, t4 * 128:(t4 + 1) * 128],
                                        identf32[:B, :B])
                    nc.vector.tensor_copy(hT[:, t4, :], tp[:])

                arin = dramp.tile([128, C // 128, B], f32)
                arout = dramp.tile([128, C // 128, B], f32)
                nc.sync.dma_start(arin[:], hT[:])
                nc.gpsimd.collective_compute(
                    "AllReduce", OP.add, replica_groups=RG,
                    ins=[arin[:]], outs=[arout[:]])
                hTr = fcsbp.tile([128, C // 128, B], f32)
                nc.sync.dma_start(hTr[:], arout[:])

                # fc2: out[d, b] = fc2_w[d, :] @ h[:, b]
                fc2wt = fcsbp.tile([128, C // 128, D], f32)
                nc.sync.dma_start(fc2wt[:], din['fc2wt'])
                fc2b = fcsbp.tile([1, D], f32)
                nc.sync.dma_start(fc2b[:], din['fc2b'])
                onesf32 = fcsbp.tile([1, B], f32)
                nc.sync.dma_start(onesf32[:], din['onesf32'])
                ps2 = fcps2p.tile([B, D], f32, tag="ps2")
                for kt in range(C // 128):
                    nc.tensor.matmul(ps2[:], hTr[:, kt, :], fc2wt[:, kt, :],
                                     start=(kt == 0), stop=False)
                nc.tensor.matmul(ps2[:], onesf32[:1, :], fc2b[:1, :],
                                 start=False, stop=True)
                sm = fcsbp.tile([B, D], f32)
                nc.vector.tensor_copy(sm[:], ps2[:])

                # log_softmax over D (free axis)
                mx = fcsbp.tile([B, 1], f32)
                nc.vector.tensor_reduce(mx[:], sm[:], AX.X, OP.max)
                xm = fcsbp.tile([B, D], f32)
                nc.vector.tensor_single_scalar(xm[:], sm[:], mx[:], OP.subtract)
                ex = fcsbp.tile([B, D], f32)
                nc.scalar.activation(ex[:], xm[:], AT.Exp)
                sume = fcsbp.tile([B, 1], f32)
                nc.vector.tensor_reduce(sume[:], ex[:], AX.X, OP.add)
                lse = fcsbp.tile([B, 1], f32)
                nc.scalar.activation(lse[:], sume[:], AT.Ln)
                res = fcsbp.tile([B, D], f32)
                nc.vector.tensor_single_scalar(res[:], xm[:], lse[:],
                                               OP.subtract)
                nc.sync.dma_start(dout, res[:])
            accp.__exit__(None, None, None)

    return nc


def _run(cfg, inputs, trace=False):
    in_maps = _host_prep(cfg, **inputs)
    nc = _build(cfg)
    nc.compile()
    from concourse import bass_utils
    res = bass_utils.run_bass_kernel_spmd(
        nc, in_maps, core_ids=list(range(cfg.NCORES)), trace=trace)
    return np.asarray(res.results[0]['out'], np.float32).copy(), res


def kernel(**inputs):
    out, _ = _run(CFG(), inputs)
    return out


# revision 9
# speedup vs baseline: 1.0370x; 1.0194x over previous
"""Trainium2 Bass kernel for NetTGCN (gnn_message_passing) — v2.

The Chebyshev SpMMs are dense matmuls against a densified normalized
adjacency LhatT (fp16, built on host from edge_index).  LhatT is
column-sharded across the 8 cores: each core holds lhsT [N, N/8] = 16 MB
SBUF-resident and computes its 1024 output rows per application.

v2 restructure vs v1:
 * Column-major SpMM: the z state tile [128 src-nodes, F<=128] is the
   STATIONARY matmul operand (one weight load per contraction tile) and
   the resident LhatT is the MOVING operand (2x512-wide).  The SpMM
   output is zT [F, 1024] which feeds the per-tap weight GEMMs directly
   (features already on partitions) — no per-tap transposes on the tap
   GEMM path and 8x fewer LDWEIGHTS on the SpMM path.
 * Two independent batch-split chains per conv (8 batches each).  Each
   chain's AllGather flies while the other chain's SpMM runs on the PE.
 * Gathered z is streamed from the DRAM gather buffer in [128, 8, F]
   super-tiles (double buffered) instead of a full SBUF copy.
 * The node-major copy of T_k needed for the gather is rebuilt with 8 PE
   transposes per chain/chunk/tap into one PSUM bank.

The FFT (real part of a length-15 DFT) is a constant cosine matrix folded
into W1 on the host.  fc1 is contraction-sharded (each core streams 1/8
of the weight from HBM) with a final AllReduce; fc2 + log_softmax run
replicated.  All matmul operands fp16 with fp32 PSUM accumulation.
"""

import numpy as np

# ---------------------------------------------------------------- config

class CFG:
    N = 8192          # nodes
    B = 16            # batch
    T = 15            # time taps
    KCH = 25          # chebyshev order
    G1 = 32
    G2 = 64
    C = 512           # fc1 out
    D = 6             # classes
    NCORES = 8
    NCH = 2           # batch chains
    PHASES = 3        # 1=conv1, 2=+conv2, 3=+fc (debug bisect)
    DEBUG = False

    @property
    def NLOC(self):
        return self.N // self.NCORES

    @property
    def MT(self):
        return self.NLOC // 128

    @property
    def KT(self):
        return self.N // 128

    @property
    def BCH(self):
        return self.B // self.NCH        # batches per chain (8)

    @property
    def F1(self):
        return self.BCH * self.T         # conv1 chain width (120)

    @property
    def F2(self):
        return self.BCH * self.G1        # conv2 chain width (256)


def _host_prep(cfg, x, edge_index, W1, b1, W2, b2, fc1_w, fc1_b, fc2_w, fc2_b):
    """Pure layout / format preprocessing -> per-core input maps."""
    f16 = np.float16
    N, B, T, K = cfg.N, cfg.B, cfg.T, cfg.KCH
    NC, NLOC, MT, KT = cfg.NCORES, cfg.NLOC, cfg.MT, cfg.KT
    G1, G2 = cfg.G1, cfg.G2

    row = np.asarray(edge_index[0], dtype=np.int64)
    col = np.asarray(edge_index[1], dtype=np.int64)
    deg = np.bincount(row, minlength=N).astype(np.float32)
    dinv = np.where(deg > 0, 1.0 / np.sqrt(np.maximum(deg, 1.0)), 0.0).astype(np.float32)
    vals = -(dinv[row] * dinv[col])
    # convention: LhatT[r, c] such that out[c] += LhatT[r, c] * z[r]
    LhatT = np.zeros((N, N), np.float32)
    np.add.at(LhatT, (row, col), vals)
    LhatT = LhatT.astype(f16)

    # x -> [p, kt, (b,t)] fp16, node n = kt*128 + p
    x_n = np.ascontiguousarray(
        np.asarray(x, np.float32).transpose(1, 0, 2).reshape(KT, 128, B * T)
        .transpose(1, 0, 2)).astype(f16)

    # fold DFT-real (cosine) matrix into W1:  xf = x @ Cf ; W1f[k] = Cf @ W1[k]
    tt = np.arange(T)
    Cf = np.cos(2 * np.pi * np.outer(tt, tt) / T).astype(np.float32)
    W1f = np.einsum('ts,ksg->ktg', Cf, np.asarray(W1, np.float32))  # [K, T, G1]

    # block-diag over the 8 batches of one chain -> sbuf [F1, K, 256]
    W1blk = np.zeros((K, cfg.F1, cfg.BCH * G1), np.float32)
    for b8 in range(cfg.BCH):
        W1blk[:, b8 * T:(b8 + 1) * T, b8 * G1:(b8 + 1) * G1] = W1f
    W1blk = np.ascontiguousarray(W1blk.transpose(1, 0, 2)).astype(f16)

    # block-diag over 4 batches of one chunk -> sbuf [128, K, 256]
    W2blk = np.zeros((K, 4 * G1, 4 * G2), np.float32)
    for b4 in range(4):
        W2blk[:, b4 * G1:(b4 + 1) * G1, b4 * G2:(b4 + 1) * G2] = \
            np.asarray(W2, np.float32)
    W2blk = np.ascontiguousarray(W2blk.transpose(1, 0, 2)).astype(f16)

    b1row = np.tile(np.asarray(b1, np.float32), B)[None, :].astype(f16)   # [1, 512]
    b2row = np.tile(np.asarray(b2, np.float32), B)[None, :].astype(f16)   # [1, 1024]
    ones_col = np.ones((1, 128), f16)
    fc1b_row = np.asarray(fc1_b, np.float32)[None, :].astype(f16)         # [1, C]
    fc2_wT = np.ascontiguousarray(
        np.asarray(fc2_w, np.float32).T.reshape(cfg.C // 128, 128, cfg.D)
        .transpose(1, 0, 2))                                              # [128, C/128, D] f32
    fc2b_col = np.asarray(fc2_b, np.float32)[None, :]                     # [1, D]
    ones_f32 = np.ones((1, cfg.B), np.float32)

    wv = np.asarray(fc1_w, np.float32).reshape(cfg.C, N, G2)
    xt = np.asarray(x, np.float32).transpose(1, 0, 2)                     # [N, B, T]

    in_maps = []
    for c in range(NC):
        # LhatT column slice -> [p, kt, mt, m] fp16  (r = kt*128+p)
        lt = LhatT[:, c * NLOC:(c + 1) * NLOC]
        lt = np.ascontiguousarray(
            lt.reshape(KT, 128, MT, 128).transpose(1, 0, 2, 3))
        # local x transposed per chain: xT[ch][(b,t), n_loc] fp16
        xl = xt[c * NLOC:(c + 1) * NLOC]                                  # [NLOC, B, T]
        xT = np.ascontiguousarray(
            xl.reshape(NLOC, cfg.NCH, cfg.BCH * T).transpose(1, 2, 0)).astype(f16)
        # fc1 weight slice -> [p, jt, cc] with jt = g*MT + mt, j = jt*128 + p
        ws = wv[:, c * NLOC:(c + 1) * NLOC, :]                            # [C, NLOC, G2]
        ws = ws.reshape(cfg.C, MT, 128, G2).transpose(2, 3, 1, 0)         # [p, g, mt, C]
        ws = np.ascontiguousarray(ws.reshape(128, G2 * MT, cfg.C)).astype(f16)
        in_maps.append(dict(
            lt=lt, x_n=x_n, x_t=xT,
            w1blk=W1blk, w2blk=W2blk, b1row=b1row, b2row=b2row,
            ones16=ones_col, fc1b=fc1b_row, fc2wt=fc2_wT, fc2b=fc2b_col,
            onesf32=ones_f32, wfc=ws,
        ))
    return in_maps


def _build(cfg):
    import concourse.bass as bass
    import concourse.mybir as mybir
    import concourse.tile as tile
    from concourse import bacc
    from concourse.masks import make_identity

    f16 = mybir.dt.float16
    f32 = mybir.dt.float32
    AT = mybir.ActivationFunctionType
    OP = mybir.AluOpType
    AX = mybir.AxisListType

    N, B, T, K = cfg.N, cfg.B, cfg.T, cfg.KCH
    NC, NLOC, MT, KT = cfg.NCORES, cfg.NLOC, cfg.MT, cfg.KT
    NCH, BCH, F1, F2 = cfg.NCH, cfg.BCH, cfg.F1, cfg.F2
    G1, G2, C, D = cfg.G1, cfg.G2, cfg.C, cfg.D
    RG = [list(range(NC))]
    KTG = KT // 8                       # kt super-tile groups (8)

    nc = bacc.Bacc("TRN2", target_bir_lowering=False, debug=False,
                   num_devices=NC)

    dt_in = {
        'lt': ([128, KT, MT, 128], f16),
        'x_n': ([128, KT, B * T], f16),
        'x_t': ([NCH, F1, NLOC], f16),
        'w1blk': ([F1, K, BCH * G1], f16),
        'w2blk': ([4 * G1, K, 4 * G2], f16),
        'b1row': ([1, B * G1], f16),
        'b2row': ([1, B * G2], f16),
        'ones16': ([1, 128], f16),
        'fc1b': ([1, C], f16),
        'fc2wt': ([128, C // 128, D], f32),
        'fc2b': ([1, D], f32),
        'onesf32': ([1, B], f32),
        'wfc': ([128, G2 * MT, C], f16),
    }
    din = {k: nc.dram_tensor(k, shp, dt, kind="ExternalInput").ap()
           for k, (shp, dt) in dt_in.items()}
    dout = nc.dram_tensor("out", [B, D], f32, kind="ExternalOutput").ap()
    if cfg.DEBUG:
        dbg_h1 = nc.dram_tensor("dbg_h1", [128, MT, B * G1], f16,
                                kind="ExternalOutput").ap()
        dbg_h2 = nc.dram_tensor("dbg_h2", [128, MT, B * G2], f16,
                                kind="ExternalOutput").ap()

    with tile.TileContext(nc) as tc:
        with (
            tc.tile_pool(name="const", bufs=1) as constp,
            tc.tile_pool(name="dram", bufs=1, space="DRAM") as dramp,
        ):
            # ---------------- constants / persistent state
            LT = constp.tile([128, KT, MT, 128], f16)
            for g in range(8):
                nc.sync.dma_start(LT[:, g * 8:(g + 1) * 8],
                                  din['lt'][:, g * 8:(g + 1) * 8])
            ident16 = constp.tile([128, 128], f16)
            make_identity(nc, ident16[:])
            identf32 = constp.tile([32, 32], f32)
            make_identity(nc, identf32[:])
            ones16 = constp.tile([1, 128], f16)
            nc.sync.dma_start(ones16[:], din['ones16'])

            # DRAM gather buffers (2 tap-parity bufs per chain)
            def gbufs(name, fdim):
                gis = [dramp.tile([128, MT * fdim], f16, name=f"{name}i{i}")
                       for i in range(2)]
                gos = [dramp.tile([NC, 128, MT, fdim], f16, name=f"{name}o{i}")
                       for i in range(2)]
                return gis, gos

            g1 = [gbufs(f"g1c{ch}", F1) for ch in range(NCH)]
            g2 = [gbufs(f"g2c{ch}", F2) for ch in range(NCH)]
            gh1_i = [dramp.tile([128, MT * BCH * G1], f16, name=f"gh1i{ch}")
                     for ch in range(NCH)]
            gh1_o = [dramp.tile([NC, 128, MT, BCH * G1], f16, name=f"gh1o{ch}")
                     for ch in range(NCH)]

            # persistent relu'd conv outputs
            accp = tc.tile_pool(name="accp", bufs=1)
            accpp = accp.__enter__()
            h1loc = accpp.tile([128, MT, B * G1], f16)
            acc2 = accpp.tile([128, MT, B * G2], f16)

            # =========================================================
            # generic chebyshev conv driver (column-major SpMM)
            # =========================================================
            def conv(tag, nq, chF, wblk, brow, zsrc_fn, ztinit_fn,
                     acc, acc_col0_fn, gbuf, pools):
                """One Chebyshev conv: NCH chains x nq chunks of width chF.

                wblk: SBUF AP [chF(+), K, 256]; tap k chunk rhs = wblk[:chF, k, :]
                zsrc_fn(kk, ch, g): DRAM AP [128, 8, chF*nq] = SpMM input
                    super-tile (T_{kk-1} node-major, contraction group g).
                ztinit_fn(ch, q, zt): init zt [chF, NLOC] = chunk of T_0^T.
                acc_col0_fn(ch, q): starting acc column of chunk (ch, q).
                """
                zsp, pszp, pstp, psgp, ztp, curp = pools
                OW = 256
                FW = chF * nq                   # chain width
                zt = [[[ztp.tile([chF, NLOC], f16,
                                 name=f"zt{tag}_{ch}_{q}_{par}")
                        for par in range(2)] for q in range(nq)]
                      for ch in range(NCH)]
                for ch in range(NCH):
                    for q in range(nq):
                        ztinit_fn(ch, q, zt[ch][q][0])

                # k = 0 tap GEMM from the initial state
                for ch in range(NCH):
                    for q in range(nq):
                        c0 = acc_col0_fn(ch, q)
                        for m2 in range(MT // 2):
                            pg = psgp.tile([128, 2, OW], f32, tag="pg",
                                           name=f"pg{tag}0_{ch}_{q}_{m2}")
                            for i in range(2):
                                mt = 2 * m2 + i
                                nc.tensor.matmul(
                                    pg[:, i, :],
                                    zt[ch][q][0][:, mt * 128:(mt + 1) * 128],
                                    wblk[:chF, 0, :], start=True, stop=True)
                            nc.vector.tensor_tensor(
                                acc[:, 2 * m2:2 * m2 + 2, c0:c0 + OW],
                                acc[:, 2 * m2:2 * m2 + 2, c0:c0 + OW],
                                pg[:], OP.add)

                for kk in range(1, K):
                    par, prev = kk % 2, (kk - 2) % 2
                    for ch in range(NCH):
                        # ---- SpMM: psT[f, c] = sum_r z[r, f] * LhatT[r, c]
                        psts = [pszp.tile([chF, MT * 128], f32, tag="psz",
                                           name=f"psz{tag}_{kk}_{ch}_{q}")
                                for q in range(nq)]
                        for g in range(KTG):
                            zs = zsp.tile([128, 8, FW], f16, tag="zs")
                            nc.scalar.dma_start(zs[:], zsrc_fn(kk, ch, g))
                            for k8 in range(8):
                                kt = g * 8 + k8
                                for q in range(nq):
                                    lhs = zs[:, k8, q * chF:(q + 1) * chF]
                                    nc.tensor.matmul(
                                        psts[q][:, 0:512], lhs,
                                        LT[:, kt, 0:4, :],
                                        start=(kt == 0), stop=(kt == KT - 1))
                                    nc.tensor.matmul(
                                        psts[q][:, 512:1024], lhs,
                                        LT[:, kt, 4:8, :],
                                        start=(kt == 0), stop=(kt == KT - 1))
                        # ---- evict: zt_k = 2*psT - zt_{k-2}   (T_1 = psT)
                        for q in range(nq):
                            dst = zt[ch][q][par]
                            if kk == 1:
                                nc.vector.tensor_copy(dst[:], psts[q][:])
                            else:
                                nc.vector.scalar_tensor_tensor(
                                    dst[:], psts[q][:], 2.0,
                                    zt[ch][q][prev][:], OP.mult, OP.subtract)
                        # ---- rebuild node-major + gather (skip last tap)
                        if kk < K - 1:
                            cur = curp.tile([128, MT, FW], f16, tag="cur")
                            for q in range(nq):
                                pt = pstp.tile([128, MT, chF], f16, tag="pst")
                                for mt in range(MT):
                                    nc.tensor.transpose(
                                        pt[:, mt, :],
                                        zt[ch][q][par][:, mt * 128:(mt + 1) * 128],
                                        ident16[:chF, :chF])
                                nc.vector.tensor_copy(
                                    cur[:, :, q * chF:(q + 1) * chF], pt[:])
                            gi, go = gbuf[ch][0][kk % 2], gbuf[ch][1][kk % 2]
                            nc.sync.dma_start(
                                gi[:].rearrange("p (m f) -> p m f", m=MT),
                                cur[:])
                            nc.gpsimd.collective_compute(
                                "AllGather", OP.bypass, replica_groups=RG,
                                ins=[gi[:]], outs=[go[:]])
                        # ---- tap GEMMs
                        last = (kk == K - 1)
                        for q in range(nq):
                            c0 = acc_col0_fn(ch, q)
                            for m2 in range(MT // 2):
                                pg = psgp.tile([128, 2, OW], f32, tag="pg",
                                               name=f"pg{tag}_{kk}_{ch}_{q}_{m2}")
                                for i in range(2):
                                    mt = 2 * m2 + i
                                    nc.tensor.matmul(
                                        pg[:, i, :],
                                        zt[ch][q][par][:, mt * 128:(mt + 1) * 128],
                                        wblk[:chF, kk, :], start=True,
                                        stop=not last)
                                    if last:
                                        nc.tensor.matmul(
                                            pg[:, i, :], ones16[:1, :128],
                                            brow[:1, c0:c0 + OW],
                                            start=False, stop=True)
                                nc.vector.tensor_tensor(
                                    acc[:, 2 * m2:2 * m2 + 2, c0:c0 + OW],
                                    acc[:, 2 * m2:2 * m2 + 2, c0:c0 + OW],
                                    pg[:], OP.add)

            # =========================================================
            # conv1: 2 chains x 1 chunk of F1=120
            # =========================================================
            with (
                tc.tile_pool(name="c1sb", bufs=1) as c1sbp,
                tc.tile_pool(name="zs1", bufs=2) as zs1p,
                tc.tile_pool(name="zt1", bufs=1) as zt1p,
                tc.tile_pool(name="cur1", bufs=2) as cur1p,
                tc.tile_pool(name="psz1", bufs=2, space="PSUM") as psz1p,
                tc.tile_pool(name="pst1", bufs=2, space="PSUM") as pst1p,
                tc.tile_pool(name="psg1", bufs=2, space="PSUM") as psg1p,
            ):
                w1 = c1sbp.tile([F1, K, BCH * G1], f16)
                nc.sync.dma_start(w1[:], din['w1blk'])
                b1row = c1sbp.tile([1, B * G1], f16)
                nc.sync.dma_start(b1row[:], din['b1row'])
                acc1 = c1sbp.tile([128, MT, B * G1], f32)
                nc.vector.memset(acc1[:], 0.0)

                def zsrc1(kk, ch, g):
                    if kk == 1:   # T_0 = x, node-major from x_n input
                        return din['x_n'][:, g * 8:(g + 1) * 8,
                                          ch * F1:(ch + 1) * F1]
                    go = g1[ch][1][(kk - 1) % 2]
                    return go[g]

                def ztinit1(ch, q, ztile):
                    nc.sync.dma_start(ztile[:], din['x_t'][ch])

                with nc.named_scope("conv1"):
                    conv("c1", 1, F1, w1, b1row, zsrc1, ztinit1,
                         acc1, lambda ch, q: ch * (BCH * G1), g1,
                         (zs1p, psz1p, pst1p, psg1p, zt1p, cur1p))

                    # h1 = relu(acc1), per-chain gather to all cores
                    HW1 = BCH * G1
                    for ch in range(NCH):
                        cs = slice(ch * HW1, (ch + 1) * HW1)
                        nc.vector.tensor_scalar_max(
                            h1loc[:, :, cs], acc1[:, :, cs], 0.0)
                        nc.sync.dma_start(
                            gh1_i[ch][:].rearrange("p (m f) -> p m f", m=MT),
                            h1loc[:, :, cs])
                        nc.gpsimd.collective_compute(
                            "AllGather", OP.bypass, replica_groups=RG,
                            ins=[gh1_i[ch][:]], outs=[gh1_o[ch][:]])
            if cfg.DEBUG:
                nc.sync.dma_start(dbg_h1, h1loc[:])

            if cfg.PHASES < 2:
                zz = constp.tile([B, D], f32)
                nc.vector.memset(zz[:], 0.0)
                nc.sync.dma_start(dout, zz[:])
                accp.__exit__(None, None, None)
                return nc

            # =========================================================
            # conv2: 2 chains x 2 chunks of 128
            # =========================================================
            with (
                tc.tile_pool(name="c2sb", bufs=1) as c2sbp,
                tc.tile_pool(name="zs2", bufs=2) as zs2p,
                tc.tile_pool(name="zt2", bufs=1) as zt2p,
                tc.tile_pool(name="cur2", bufs=2) as cur2p,
                tc.tile_pool(name="psz2", bufs=2, space="PSUM") as psz2p,
                tc.tile_pool(name="pst2", bufs=2, space="PSUM") as pst2p,
                tc.tile_pool(name="psg2", bufs=2, space="PSUM") as psg2p,
            ):
                w2 = c2sbp.tile([4 * G1, K, 4 * G2], f16)
                nc.sync.dma_start(w2[:], din['w2blk'])
                b2row = c2sbp.tile([1, B * G2], f16)
                nc.sync.dma_start(b2row[:], din['b2row'])
                nc.vector.memset(acc2[:], 0.0)

                def zsrc2(kk, ch, g):
                    if kk == 1:   # T_0 = h1 full, from the per-chain gather
                        return gh1_o[ch][g]
                    go = g2[ch][1][(kk - 1) % 2]
                    return go[g]

                def ztinit2(ch, q, ztile):
                    # zt = (h1 chunk)^T via PE transposes of h1loc columns
                    f0 = ch * F2 + q * 128
                    pt = pst2p.tile([128, MT, 128], f16, tag="pst")
                    for mt in range(MT):
                        nc.tensor.transpose(
                            pt[:, mt, :], h1loc[:, mt, f0:f0 + 128],
                            ident16[:])
                    nc.vector.tensor_copy(
                        ztile[:].rearrange("p (m f) -> p m f", m=MT), pt[:])

                with nc.named_scope("conv2"):
                    conv("c2", 2, 128, w2, b2row, zsrc2, ztinit2,
                         acc2, lambda ch, q: ch * (BCH * G2) + q * 256, g2,
                         (zs2p, psz2p, pst2p, psg2p, zt2p, cur2p))
                    nc.vector.tensor_scalar_max(acc2[:], acc2[:], 0.0)
            if cfg.DEBUG:
                nc.sync.dma_start(dbg_h2, acc2[:])

            if cfg.PHASES < 3:
                zz = constp.tile([B, D], f32)
                nc.vector.memset(zz[:], 0.0)
                nc.sync.dma_start(dout, zz[:])
                accp.__exit__(None, None, None)
                return nc

            # =========================================================
            # fc1 (streamed weights, contraction-sharded) + fc2 + lsm
            # =========================================================
            h2v = acc2[:].rearrange("p m (b g) -> p m b g", b=B)
            with (
                nc.named_scope("fc"),
                tc.tile_pool(name="fcw", bufs=3) as fcwp,
                tc.tile_pool(name="fcps", bufs=1, space="PSUM") as fcpsp,
                tc.tile_pool(name="fcsb", bufs=1) as fcsbp,
                tc.tile_pool(name="fcps2", bufs=2, space="PSUM") as fcps2p,
            ):
                JT = G2 * MT            # 512 j-tiles
                JBLK = 8
                psfc = fcpsp.tile([B, C], f32)
                fc1b_sb = fcsbp.tile([1, C], f16)
                nc.sync.dma_start(fc1b_sb[:], din['fc1b'])
                for jb in range(JT // JBLK):
                    wbuf = fcwp.tile([128, JBLK, C], f16, tag="wbuf")
                    nc.sync.dma_start(
                        wbuf[:], din['wfc'][:, jb * JBLK:(jb + 1) * JBLK, :])
                    for ji in range(JBLK):
                        jt = jb * JBLK + ji
                        g, mt = jt // MT, jt % MT
                        nc.tensor.matmul(psfc[:], h2v[:, mt, :, g],
                                         wbuf[:, ji, :],
                                         start=(jt == 0), stop=False)
                nc.tensor.matmul(psfc[:], ones16[:1, :B], fc1b_sb[:1, :],
                                 start=False, stop=True)

                # transpose [B, C] -> [128, C/128, B]
                hsb = fcsbp.tile([B, C], f32)
                nc.vector.tensor_copy(hsb[:], psfc[:])
                hT = fcsbp.tile([128, C // 128, B], f32)
                for t4 in range(C // 128):
                    tp = fcps2p.tile([128, B], f32, tag="fct")
                    nc.tensor.transpose(tp[:], hsb[:, t4 * 128:(t4 + 1) * 128],
                                        identf32[:B, :B])
                    nc.vector.tensor_copy(hT[:, t4, :], tp[:])

                arin = dramp.tile([128, C // 128, B], f32)
                arout = dramp.tile([128, C // 128, B], f32)
                nc.sync.dma_start(arin[:], hT[:])
                nc.gpsimd.collective_compute(
                    "AllReduce", OP.add, replica_groups=RG,
                    ins=[arin[:]], outs=[arout[:]])
                hTr = fcsbp.tile([128, C // 128, B], f32)
                nc.sync.dma_start(hTr[:], arout[:])

                # fc2: out[d, b] = fc2_w[d, :] @ h[:, b]
                fc2wt = fcsbp.tile([128, C // 128, D], f32)
                nc.sync.dma_start(fc2wt[:], din['fc2wt'])
                fc2b = fcsbp.tile([1, D], f32)
                nc.sync.dma_start(fc2b[:], din['fc2b'])
                onesf32 = fcsbp.tile([1, B], f32)
                nc.sync.dma_start(onesf32[:], din['onesf32'])
                ps2 = fcps2p.tile([B, D], f32, tag="ps2")
                for kt in range(C // 128):
                    nc.tensor.matmul(ps2[:], hTr[:, kt, :], fc2wt[:, kt, :],
                                     start=(kt == 0), stop=False)
                nc.tensor.matmul(ps2[:], onesf32[:1, :], fc2b[:1, :],
                                 start=False, stop=True)
                sm = fcsbp.tile([B, D], f32)
                nc.vector.tensor_copy(sm[:], ps2[:])

                # log_softmax over D (free axis)
                mx = fcsbp.tile([B, 1], f32)
                nc.vector.tensor_reduce(mx[:], sm[:], AX.X, OP.max)
                xm = fcsbp.tile([B, D], f32)
                nc.vector.tensor_single_scalar(xm[:], sm[:], mx[:], OP.subtract)
                ex = fcsbp.tile([B, D], f32)
                nc.scalar.activation(ex[:], xm[:], AT.Exp)
                sume = fcsbp.tile([B, 1], f32)
                nc.vector.tensor_reduce(sume[:], ex[:], AX.X, OP.add)
                lse = fcsbp.tile([B, 1], f32)
                nc.scalar.activation(lse[:], sume[:], AT.Ln)
                res = fcsbp.tile([B, D], f32)
                nc.vector.tensor_single_scalar(res[:], xm[:], lse[:],
                                               OP.subtract)
                nc.sync.dma_start(dout, res[:])
            accp.__exit__(None, None, None)

    return nc


def _run(cfg, inputs, trace=False):
    in_maps = _host_prep(cfg, **inputs)
    nc = _build(cfg)
    nc.compile()
    from concourse import bass_utils
    res = bass_utils.run_bass_kernel_spmd(
        nc, in_maps, core_ids=list(range(cfg.NCORES)), trace=trace)
    return np.asarray(res.results[0]['out'], np.float32).copy(), res


def kernel(**inputs):
    out, _ = _run(CFG(), inputs)
    return out
